# revision 25
# baseline (speedup 1.0000x reference)
"""DEVISE margin hinge loss on 8 Trainium2 NeuronCores (Bass/Tile).

Data-parallel: batch sharded 8 ways, weights + label embeddings replicated.
Per core: one matmul chain produces projT duplicated into both PSUM halves
(W columns pre-duplicated at pack time); true-sim via elementwise mult +
ones-matmul partition reduction; hinge sweep tiles sims into (128,1024)
PSUM slots consumed by ACT (relu+bias+accum fused, 11 slots per m-chunk)
and DVE (scalar_tensor_tensor+accum, 9 slots) to match their 1.2 : 0.96
GHz rates. Loads spread over 4 engine HWDGE queues so DMA overlaps the
sweep. Partial scalar per core; host sums and normalizes.
"""

import numpy as np

B, D, C, DC = 4096, 1024, 20000, 64
MARGIN = 0.1
NCORES = 8
BL = B // NCORES           # 512 local batch
M_CHUNKS = BL // 128       # 4
K_CHUNKS = D // 128        # 8
CP_LO = 10240              # classes in partitions 0:64 of et
CP_HI = C - CP_LO          # 9760 classes in partitions 64:128
ET_TILES = 5               # (128, 2048) SBUF tiles covering et
G_PER_M = 10               # 1024-wide pair groups per m-chunk sweep
HI_LAST = CP_HI - 9 * 1024  # 544: width of the last hi slot
DVE_LO = (0, 5)            # the one lo slot routed to DVE (39:41 balance)
N_A = M_CHUNKS * G_PER_M - 1   # 39 ACT slots
N_D = M_CHUNKS * G_PER_M + 1   # 41 DVE slots

_cache = {}


def _build_nc(reps: int = 1, variant: str = "full"):
    import concourse.bacc as bacc
    import concourse.mybir as mybir
    import concourse.tile as tile

    dt = mybir.dt.float32
    bf = mybir.dt.bfloat16
    Act = mybir.ActivationFunctionType
    Alu = mybir.AluOpType

    nc = bacc.Bacc()
    xt_d = nc.declare_dram_parameter("xt", [128, K_CHUNKS * BL], bf, isOutput=False)
    w2_d = nc.declare_dram_parameter("w2", [128, K_CHUNKS * 128], bf, isOutput=False)
    et_d = nc.declare_dram_parameter("et", [128, ET_TILES * 2048], bf, isOutput=False)
    eyt_d = nc.declare_dram_parameter("eyt", [128, BL], dt, isOutput=False)
    outa_d = nc.declare_dram_parameter("outa", [128, N_A], dt, isOutput=True)
    outd_d = nc.declare_dram_parameter("outd", [128, N_D], dt, isOutput=True)

    with tile.TileContext(nc) as tc:
        def body(_iv=None):
            with tc.tile_pool(name="const", bufs=1) as cpool:
                # ---- loads spread over 4 HWDGE queues ---------------------
                # load order matters: the DMA engines drain one transfer at a
                # time in enqueue order, so phase-1 inputs (w2, xt, eyt) go
                # first across both HWDGE queues, then the et stream.
                w2_sb = cpool.tile([128, K_CHUNKS * 128], bf, tag="w2")
                et_sb = []
                for i in range(ET_TILES):
                    t = cpool.tile([128, 2048], bf, tag=f"et{i}")
                    et_sb.append(t)
                xt_sb = cpool.tile([128, K_CHUNKS * BL], bf, tag="xt")
                eyt_sb = cpool.tile([128, BL], dt, tag="eyt")
                nc.scalar.dma_start(w2_sb[:], w2_d[:])
                for h in range(2):
                    nc.sync.dma_start(
                        xt_sb[:, h * 4 * BL : (h + 1) * 4 * BL],
                        xt_d[:, h * 4 * BL : (h + 1) * 4 * BL],
                    )
                nc.scalar.dma_start(eyt_sb[:], eyt_d[:])
                nc.scalar.dma_start(et_sb[0][:], et_d[:, 0:2048])
                nc.sync.dma_start(et_sb[1][:], et_d[:, 2048:4096])
                nc.scalar.dma_start(et_sb[2][:], et_d[:, 4096:6144])
                nc.sync.dma_start(et_sb[3][:], et_d[:, 6144:8192])
                nc.scalar.dma_start(et_sb[4][:], et_d[:, 8192:10240])

                projT = cpool.tile([128, BL], bf, tag="projT")
                tmul = cpool.tile([128, BL], dt, tag="tmul")
                bias_col = cpool.tile([128, M_CHUNKS], dt, tag="bias")
                ones_col = cpool.tile([128, 1], dt, tag="ones")
                nc.vector.memset(ones_col[:], 1.0)
                # every stats column is written by exactly one accum_out
                # (overwrite semantics), so no zeroing needed
                stats_a = cpool.tile([128, N_A], dt, tag="stats_a")
                stats_d = cpool.tile([128, N_D], dt, tag="stats_d")
                zeros = cpool.tile([128, 1024], dt, tag="zeros")
                nc.vector.memset(zeros[:], 0.0)
                wones = cpool.tile([128, 64], bf, tag="wones")
                nc.vector.memset(wones[:], 1.0)
                # single-buffer scratch, each written by exactly one engine
                a_scr = cpool.tile([128, 1024], dt, tag="ascr")
                d_scr = cpool.tile([128, 1024], dt, tag="dscr")

                if variant == "dma":
                    with tc.tile_pool(name="pdma", bufs=1, space="PSUM") as pd:
                        for t in [*et_sb, xt_sb, w2_sb, eyt_sb]:
                            tt = pd.tile([1, 1], dt, tag="touch")
                            nc.tensor.matmul(
                                tt[:], t[:, 0:1], t[:, 0:1], start=True, stop=True
                            )
                        nc.vector.memset(stats_a[:], 0.0)
                        nc.vector.memset(stats_d[:], 0.0)
                        nc.sync.dma_start(outa_d[:], stats_a[:])
                        nc.scalar.dma_start(outd_d[:], stats_d[:])
                    return

                # ---- phase 1: projT (both halves in one chain) + bias -----
                with tc.tile_pool(name="ppre", bufs=1, space="PSUM") as ppre:
                    # warm the PE clock gate during the DMA wait: ~4us of
                    # dummy matmuls with no load dependencies, written into
                    # psum_pr's region (the chain's start=True overwrites)
                    psum_pr = ppre.tile([128, BL], dt, tag="pp")
                    for _ in range(40):
                        nc.tensor.matmul(
                            psum_pr[0:64, 0:64],
                            wones[:],
                            wones[:],
                            start=True,
                            stop=True,
                        )
                    for k in range(K_CHUNKS):
                        nc.tensor.matmul(
                            psum_pr[:],
                            w2_sb[:, k * 128 : (k + 1) * 128],
                            xt_sb[:, k * BL : (k + 1) * BL],
                            start=(k == 0),
                            stop=(k == K_CHUNKS - 1),
                        )
                    # copy first: projT (bf16) is then the sole source for
                    # the t computation, freeing psum_pr for phase-2 reuse
                    nc.scalar.copy(projT[:], psum_pr[:])
                    psum_t = ppre.tile([128, M_CHUNKS], dt, tag="pt")
                    nc.vector.tensor_mul(tmul[:], projT[:], eyt_sb[:])
                    for m in range(M_CHUNKS):
                        nc.tensor.matmul(
                            psum_t[:, m : m + 1],
                            tmul[:, m * 128 : (m + 1) * 128],
                            ones_col[:],
                            start=True,
                            stop=True,
                        )
                    nc.vector.tensor_scalar(
                        bias_col[:], psum_t[:], -0.5, MARGIN,
                        op0=Alu.mult, op1=Alu.add,
                    )

                # ---- phase 2: hinge sweep ---------------------------------
                it_a = 0
                it_d = 0
                with (
                    tc.tile_pool(name="pa", bufs=2, space="PSUM") as pa,
                    tc.tile_pool(name="pd", bufs=2, space="PSUM") as pd2,
                ):
                    for m in range(M_CHUNKS):
                        bias_m = bias_col[:, m : m + 1]
                        lhs_lo = projT[0:64, m * 128 : (m + 1) * 128]
                        lhs_hi = projT[64:128, m * 128 : (m + 1) * 128]
                        for g in range(G_PER_M):
                            eti, off = divmod(g, 2)
                            cs = off * 1024
                            # lo half (512-col matmuls: one PSUM bank each);
                            # usually ACT, one slot per run to DVE for balance
                            lo_to_dve = (m, g) == DVE_LO
                            slot_a = (pd2 if lo_to_dve else pa).tile(
                                [128, 1024], dt, tag="pd" if lo_to_dve else "pa"
                            )
                            for s in range(2):
                                nc.tensor.matmul(
                                    slot_a[:, s * 512 : (s + 1) * 512],
                                    lhs_lo,
                                    et_sb[eti][0:64, cs + s * 512 : cs + (s + 1) * 512],
                                    start=True,
                                    stop=True,
                                    tile_position=(0, 0),
                                )
                            if variant != "nocons":
                                if lo_to_dve:
                                    nc.vector.scalar_tensor_tensor(
                                        out=d_scr[:],
                                        in0=slot_a[:],
                                        scalar=bias_m,
                                        in1=zeros[:],
                                        op0=Alu.add,
                                        op1=Alu.max,
                                        accum_out=stats_d[:, it_d : it_d + 1],
                                    )
                                    it_d += 1
                                else:
                                    nc.scalar.activation(
                                        a_scr[:], slot_a[:], Act.Relu,
                                        bias=bias_m, scale=1.0,
                                        accum_out=stats_a[:, it_a : it_a + 1],
                                    )
                                    it_a += 1
                            # hi half -> DVE
                            w = HI_LAST if g == G_PER_M - 1 else 1024
                            slot_h = pd2.tile([128, 1024], dt, tag="pd")
                            for s0 in range(0, w, 512):
                                sw = min(512, w - s0)
                                nc.tensor.matmul(
                                    slot_h[:, s0 : s0 + sw],
                                    lhs_hi,
                                    et_sb[eti][64:128, cs + s0 : cs + s0 + sw],
                                    start=True,
                                    stop=True,
                                    tile_position=(64, 0),
                                )
                            if variant != "nocons":
                                nc.vector.scalar_tensor_tensor(
                                    out=d_scr[:, 0:w],
                                    in0=slot_h[:, 0:w],
                                    scalar=bias_m,
                                    in1=zeros[:, 0:w],
                                    op0=Alu.add,
                                    op1=Alu.max,
                                    accum_out=stats_d[:, it_d : it_d + 1],
                                )
                                it_d += 1

                # ---- phase 3: ship raw per-slot stats; host reduces -------
                nc.sync.dma_start(outa_d[:], stats_a[:])
                nc.scalar.dma_start(outd_d[:], stats_d[:])

        if reps == 1:
            body()
        else:
            with tc.For_i(0, reps, 1) as iv:
                body(iv)

    nc.finalize()
    return nc


def _pack_inputs(X, y, E, W):
    """Per-core DRAM images. Layouts match the device program above."""
    import ml_dtypes

    bf16 = ml_dtypes.bfloat16
    X = np.ascontiguousarray(np.asarray(X, dtype=np.float32))
    y = np.asarray(y).astype(np.int64)
    E = np.ascontiguousarray(np.asarray(E, dtype=np.float32))
    W = np.ascontiguousarray(np.asarray(W, dtype=np.float32))

    # w2: per k-chunk, W[k] duplicated along columns -> [128, 8*128]
    w2_pack = np.concatenate(
        [np.concatenate([W[k * 128 : (k + 1) * 128]] * 2, axis=1) for k in range(K_CHUNKS)],
        axis=1,
    ).astype(bf16)
    w2_pack = np.ascontiguousarray(w2_pack)
    Et = E.T  # (64, C)
    et_pack = np.zeros((128, CP_LO), dtype=np.float32)
    et_pack[:64, :] = Et[:, :CP_LO]
    et_pack[64:, :CP_HI] = Et[:, CP_LO:]
    et_pack = np.ascontiguousarray(et_pack.astype(bf16))

    in_maps = []
    for s in range(NCORES):
        Xs = X[s * BL : (s + 1) * BL]  # (BL, D)
        xt_pack = np.ascontiguousarray(
            Xs.T.reshape(K_CHUNKS, 128, BL).transpose(1, 0, 2).reshape(128, K_CHUNKS * BL)
        ).astype(bf16)
        EyT = E[y[s * BL : (s + 1) * BL]].T  # (DC, BL)
        eyt_pack = np.ascontiguousarray(np.concatenate([EyT, EyT], axis=0))
        in_maps.append({"xt": xt_pack, "w2": w2_pack, "et": et_pack, "eyt": eyt_pack})
    return in_maps


def run_spmd(in_maps, reps: int = 1, trace: bool = False):
    from concourse.bass_utils import run_bass_kernel_spmd

    key = reps
    if key not in _cache:
        _cache[key] = _build_nc(reps)  # full variant only
    nc = _cache[key]
    return run_bass_kernel_spmd(
        nc, in_maps, core_ids=list(range(NCORES)), trace=trace
    )


def kernel(X, y, label_embeddings, weights):
    in_maps = _pack_inputs(X, y, label_embeddings, weights)
    res = run_spmd(in_maps).results
    total = sum(
        float(res[s]["outa"].sum()) + float(res[s]["outd"].sum())
        for s in range(NCORES)
    )
    loss = np.float32(total / B - MARGIN)
    return np.array([loss], dtype=np.float32)


# revision 33
# speedup vs baseline: 1.1638x; 1.1638x over previous
"""DEVISE margin hinge loss on 8 Trainium2 NeuronCores (Bass/Tile).

Data-parallel: batch sharded 8 ways, weights + label embeddings replicated.
Per core: one matmul chain produces projT duplicated into both PSUM halves
(W columns pre-duplicated at pack time); true-sim via elementwise mult +
ones-matmul partition reduction; hinge sweep tiles sims into (128,1024)
PSUM slots consumed by ACT (relu+bias+accum fused) and DVE
(scalar_tensor_tensor+accum) at a 39:41 split. All pools live outside the
rep loop (staggered semaphore reset, no all-engine barrier), loads stream
on the SP HWDGE queue in priority order, and the raw per-slot stats ship
out via gpsimd SWDGE so the tail never blocks the next iteration's loads.
Host sums the stats and normalizes.
"""

import numpy as np

B, D, C, DC = 4096, 1024, 20000, 64
MARGIN = 0.1
NCORES = 8
BL = B // NCORES           # 512 local batch
M_CHUNKS = BL // 128       # 4
K_CHUNKS = D // 128        # 8
CP_LO = 10240              # classes in partitions 0:64 of et
CP_HI = C - CP_LO          # 9760 classes in partitions 64:128
ET_TILES = 5               # (128, 2048) SBUF tiles covering et
G_PER_M = 10               # 1024-wide pair groups per m-chunk sweep
HI_LAST = CP_HI - 9 * 1024  # 544: width of the last hi slot
DVE_LO_SET = {(0, 5)}      # lo slots routed to DVE (ACT:DVE balance)
N_A = M_CHUNKS * G_PER_M - len(DVE_LO_SET)
N_D = M_CHUNKS * G_PER_M + len(DVE_LO_SET)

_cache = {}


def _build_nc(reps: int = 1, variant: str = "full"):
    import concourse.bacc as bacc
    import concourse.mybir as mybir
    import concourse.tile as tile

    dt = mybir.dt.float32
    bf = mybir.dt.bfloat16
    Act = mybir.ActivationFunctionType
    Alu = mybir.AluOpType

    nc = bacc.Bacc()
    xt_d = nc.declare_dram_parameter("xt", [128, K_CHUNKS * BL], bf, isOutput=False)
    w2_d = nc.declare_dram_parameter("w2", [128, K_CHUNKS * 128], bf, isOutput=False)
    et_d = nc.declare_dram_parameter("et", [128, ET_TILES * 2048], bf, isOutput=False)
    eyt_d = nc.declare_dram_parameter("eyt", [128, BL], dt, isOutput=False)
    outa_d = nc.declare_dram_parameter("outa", [128, N_A], dt, isOutput=True)
    outd_d = nc.declare_dram_parameter("outd", [128, N_D], dt, isOutput=True)

    with tile.TileContext(nc) as tc:
        with (
            tc.tile_pool(name="const", bufs=1) as cpool,
            tc.tile_pool(name="dbuf", bufs=2) as spool,
            tc.tile_pool(name="pa", bufs=2, space="PSUM") as pa,
            tc.tile_pool(name="pd", bufs=2, space="PSUM") as pd2,
        ):
            # constants + persistent input tiles: fixed addresses, set once
            ones_col = cpool.tile([128, 1], dt, tag="ones")
            nc.vector.memset(ones_col[:], 1.0)
            zeros = cpool.tile([128, 1024], dt, tag="zeros")
            nc.vector.memset(zeros[:], 0.0)
            w2_sb = cpool.tile([128, K_CHUNKS * 128], bf, tag="w2")
            xt_sb = cpool.tile([128, K_CHUNKS * BL], bf, tag="xt")
            eyt_sb = cpool.tile([128, BL], dt, tag="eyt")
            et_sb = [
                cpool.tile([128, 2048], bf, tag=f"et{i}", name=f"et{i}")
                for i in range(ET_TILES)
            ]
            tmul = cpool.tile([128, BL], dt, tag="tmul")
            # single-buffer scratch, each written by exactly one engine
            a_scr = cpool.tile([128, 1024], dt, tag="ascr")
            d_scr = cpool.tile([128, 1024], dt, tag="dscr")

            def body(_iv=None):
                # ---- loads: all on the SP HWDGE queue, priority order -----
                nc.sync.dma_start(w2_sb[:], w2_d[:])
                for h in range(2):
                    nc.sync.dma_start(
                        xt_sb[:, h * 4 * BL : (h + 1) * 4 * BL],
                        xt_d[:, h * 4 * BL : (h + 1) * 4 * BL],
                    )
                nc.sync.dma_start(eyt_sb[:], eyt_d[:])
                for i in range(ET_TILES):
                    nc.sync.dma_start(et_sb[i][:], et_d[:, i * 2048 : (i + 1) * 2048])

                # per-iteration tiles: double-buffered so iteration i+1 can
                # write while iteration i's late readers still run
                projT = spool.tile([128, BL], bf, tag="projT")
                bias_col = spool.tile([128, M_CHUNKS], dt, tag="bias")
                stats_a = spool.tile([128, N_A], dt, tag="sa")
                stats_d = spool.tile([128, N_D], dt, tag="sd")
                if variant in ("nocons", "dma"):
                    nc.vector.memset(stats_a[:], 0.0)
                    nc.vector.memset(stats_d[:], 0.0)

                if variant == "dma":
                    for t in [*et_sb, xt_sb, w2_sb, eyt_sb]:
                        tt = pa.tile([128, 1024], dt, tag="pa")
                        nc.tensor.matmul(
                            tt[0:1, 0:1], t[:, 0:1], t[:, 0:1], start=True, stop=True
                        )
                    nc.gpsimd.dma_start(outa_d[:], stats_a[:])
                    nc.gpsimd.dma_start(outd_d[:], stats_d[:])
                    return

                # ---- phase 1: projT chain + bias, inside one pa buffer ----
                # chain accumulates in cols 0:512 (bank 0); the four 1-col
                # t-matmuls land in cols 512:516 (bank 1)
                ch = pa.tile([128, 1024], dt, tag="pa")
                for k in range(K_CHUNKS):
                    nc.tensor.matmul(
                        ch[:, 0:BL],
                        w2_sb[:, k * 128 : (k + 1) * 128],
                        xt_sb[:, k * BL : (k + 1) * BL],
                        start=(k == 0),
                        stop=(k == K_CHUNKS - 1),
                    )
                nc.scalar.copy(projT[:], ch[:, 0:BL])
                nc.vector.tensor_mul(tmul[:], projT[:], eyt_sb[:])
                for m in range(M_CHUNKS):
                    nc.tensor.matmul(
                        ch[:, BL + m : BL + m + 1],
                        tmul[:, m * 128 : (m + 1) * 128],
                        ones_col[:],
                        start=True,
                        stop=True,
                    )
                nc.vector.tensor_scalar(
                    bias_col[:], ch[:, BL : BL + M_CHUNKS], -0.5, MARGIN,
                    op0=Alu.mult, op1=Alu.add,
                )

                # ---- phase 2: hinge sweep ---------------------------------
                it_a = 0
                it_d = 0
                for m in range(M_CHUNKS):
                    bias_m = bias_col[:, m : m + 1]
                    lhs_lo = projT[0:64, m * 128 : (m + 1) * 128]
                    lhs_hi = projT[64:128, m * 128 : (m + 1) * 128]
                    for g in range(G_PER_M):
                        eti, off = divmod(g, 2)
                        cs = off * 1024
                        # lo half (512-col matmuls: one PSUM bank each)
                        lo_to_dve = (m, g) in DVE_LO_SET
                        slot_a = (pd2 if lo_to_dve else pa).tile(
                            [128, 1024], dt, tag="pd" if lo_to_dve else "pa"
                        )
                        for s in range(2):
                            nc.tensor.matmul(
                                slot_a[:, s * 512 : (s + 1) * 512],
                                lhs_lo,
                                et_sb[eti][0:64, cs + s * 512 : cs + (s + 1) * 512],
                                start=True,
                                stop=True,
                                tile_position=(0, 0),
                            )
                        if variant != "nocons":
                            if lo_to_dve:
                                nc.vector.scalar_tensor_tensor(
                                    out=d_scr[:],
                                    in0=slot_a[:],
                                    scalar=bias_m,
                                    in1=zeros[:],
                                    op0=Alu.add,
                                    op1=Alu.max,
                                    accum_out=stats_d[:, it_d : it_d + 1],
                                )
                                it_d += 1
                            else:
                                nc.scalar.activation(
                                    a_scr[:], slot_a[:], Act.Relu,
                                    bias=bias_m, scale=1.0,
                                    accum_out=stats_a[:, it_a : it_a + 1],
                                )
                                it_a += 1
                        # hi half -> DVE
                        w = HI_LAST if g == G_PER_M - 1 else 1024
                        slot_h = pd2.tile([128, 1024], dt, tag="pd")
                        for s0 in range(0, w, 512):
                            sw = min(512, w - s0)
                            nc.tensor.matmul(
                                slot_h[:, s0 : s0 + sw],
                                lhs_hi,
                                et_sb[eti][64:128, cs + s0 : cs + s0 + sw],
                                start=True,
                                stop=True,
                                tile_position=(64, 0),
                            )
                        if variant != "nocons":
                            nc.vector.scalar_tensor_tensor(
                                out=d_scr[:, 0:w],
                                in0=slot_h[:, 0:w],
                                scalar=bias_m,
                                in1=zeros[:, 0:w],
                                op0=Alu.add,
                                op1=Alu.max,
                                accum_out=stats_d[:, it_d : it_d + 1],
                            )
                            it_d += 1

                # ---- phase 3: ship raw stats via SWDGE; host reduces ------
                nc.gpsimd.dma_start(outa_d[:], stats_a[:])
                nc.gpsimd.dma_start(outd_d[:], stats_d[:])

            if reps == 1:
                body()
            else:
                with tc.For_i(0, reps, 1, staggered_reset=True) as iv:
                    body(iv)

    nc.finalize()
    return nc


def _pack_inputs(X, y, E, W):
    """Per-core DRAM images. Layouts match the device program above."""
    import ml_dtypes

    bf16 = ml_dtypes.bfloat16
    X = np.ascontiguousarray(np.asarray(X, dtype=np.float32))
    y = np.asarray(y).astype(np.int64)
    E = np.ascontiguousarray(np.asarray(E, dtype=np.float32))
    W = np.ascontiguousarray(np.asarray(W, dtype=np.float32))

    # w2: per k-chunk, W[k] duplicated along columns -> [128, 8*128]
    w2_pack = np.concatenate(
        [np.concatenate([W[k * 128 : (k + 1) * 128]] * 2, axis=1) for k in range(K_CHUNKS)],
        axis=1,
    ).astype(bf16)
    w2_pack = np.ascontiguousarray(w2_pack)
    Et = E.T  # (64, C)
    et_pack = np.zeros((128, CP_LO), dtype=np.float32)
    et_pack[:64, :] = Et[:, :CP_LO]
    et_pack[64:, :CP_HI] = Et[:, CP_LO:]
    et_pack = np.ascontiguousarray(et_pack.astype(bf16))

    in_maps = []
    for s in range(NCORES):
        Xs = X[s * BL : (s + 1) * BL]  # (BL, D)
        xt_pack = np.ascontiguousarray(
            Xs.T.reshape(K_CHUNKS, 128, BL).transpose(1, 0, 2).reshape(128, K_CHUNKS * BL)
        ).astype(bf16)
        EyT = E[y[s * BL : (s + 1) * BL]].T  # (DC, BL)
        eyt_pack = np.ascontiguousarray(np.concatenate([EyT, EyT], axis=0))
        in_maps.append({"xt": xt_pack, "w2": w2_pack, "et": et_pack, "eyt": eyt_pack})
    return in_maps


def run_spmd(in_maps, reps: int = 1, trace: bool = False):
    from concourse.bass_utils import run_bass_kernel_spmd

    key = reps
    if key not in _cache:
        _cache[key] = _build_nc(reps)  # full variant only
    nc = _cache[key]
    return run_bass_kernel_spmd(
        nc, in_maps, core_ids=list(range(NCORES)), trace=trace
    )


def kernel(X, y, label_embeddings, weights):
    in_maps = _pack_inputs(X, y, label_embeddings, weights)
    res = run_spmd(in_maps).results
    total = sum(
        float(res[s]["outa"].sum()) + float(res[s]["outd"].sum())
        for s in range(NCORES)
    )
    loss = np.float32(total / B - MARGIN)
    return np.array([loss], dtype=np.float32)
